# revision 47
# baseline (speedup 1.0000x reference)
"""Trainium2 Bass kernel for ModalitySpecificLocalSelfAttention (7x7 window).

Spatial-parallel over H across 8 cores (16-row stripe + 3-row halo each).
v3 design (from v2's 147us trace; baseline was 98.6us):
  - V path: per-block conv matmuls (moving operand = strided xsp window)
    write each block's 14x22 neighborhood straight into vn16 -- no gather
    copies at all (v2 lost 12us on DVE gathers / 75us on GPSIMD norm).
  - Additive window mask folded into the S accumulation as a second
    matmul (identity stationary); exp then runs with accum_out so the
    softmax denominator z is produced by the ACT pass for free.
    DVE softmax work drops to: batched oob-add + reciprocal + normalize.
  - DMA transposes quad-batched (4 blocks per transfer) and split across
    both HWDGE rings: vt on ACT ring, at on sync ring.
  - 8-bank PSUM tenancy rotation across v-blocks/convs/attention/o-conv.
  - Conv epilogues pair-batched, distributed ACT/DVE; bf16 output DMA.
"""

import sys

for _p in ("/opt/trn_rl_repo", "/root/.axon_site/_ro/trn_rl_repo"):
    if _p not in sys.path:
        sys.path.append(_p)

import ml_dtypes
import numpy as np

import concourse.bass as bass
from concourse import mybir
from concourse.bass_utils import run_bass_kernel_spmd

F32 = mybir.dt.float32
BF16 = mybir.dt.bfloat16

C = 128
H = 128
W = 128
NCORES = 8
RPC = H // NCORES          # 16 rows per core
PAD = 3
HALO = RPC + 2 * PAD       # 22 rows incl halo
WP = W + 2 * PAD           # 134 padded width
BR, BC = 8, 16             # pixel block 8 rows x 16 cols
NR, NC_ = BR + 2 * PAD, BC + 2 * PAD   # 14 x 22 neighborhood
NN = NR * NC_              # 308
NN2 = 384                  # padded to 3x128 for the xbar transpose
NPIX = RPC * W             # 2048
NXP = HALO * WP            # 2948 padded stripe pixels
CH = 512
EXP_SHIFT = -16.0
MASKV = -40.0
DEBUG_OUTS = False

RELU = mybir.ActivationFunctionType.Relu
IDENT = mybir.ActivationFunctionType.Identity
EXP = mybir.ActivationFunctionType.Exp
ADD = mybir.AluOpType.add
MULT = mybir.AluOpType.mult
MAXOP = mybir.AluOpType.max


def _build_program():
    nc = bass.Bass("TRN2", target_bir_lowering=False, debug=False)

    # ---- DRAM I/O ----
    xs_d = nc.dram_tensor("xs", [C, NXP], BF16, kind="ExternalInput").ap()
    wall_d = nc.dram_tensor("wall", [C, 8 * C], BF16, kind="ExternalInput").ap()
    ball_d = nc.dram_tensor("ball", [C, 8], F32, kind="ExternalInput").ap()
    smask_d = nc.dram_tensor("smask", [C, 16, NN], BF16,
                             kind="ExternalInput").ap()
    oobc_d = nc.dram_tensor("oobc", [C, 16], F32, kind="ExternalInput").ap()
    y_d = nc.dram_tensor("y", [C, NPIX], BF16, kind="ExternalOutput").ap()

    # ---- SBUF ----
    sb = lambda name, shape, dt: nc.alloc_sbuf_tensor(name, list(shape), dt).ap()
    xsp = sb("xsp_sb", [C, HALO, WP], BF16)
    k1 = sb("k1_sb", [C, HALO * W], BF16)
    q1 = sb("q1_sb", [C, NPIX], BF16)
    q = sb("q_sb", [C, 16, C], BF16)       # block-major
    kpad = sb("kpad_sb", [C, HALO, WP], BF16)
    vn16 = sb("vn16_sb", [C, 16, NN2], BF16)
    vt16 = sb("vt16_sb", [C, 16, 3, C], BF16)
    e8 = sb("e8_sb", [C, 8, NN], BF16)
    am8 = sb("am8_sb", [C, 8, NN2], BF16)
    at8 = sb("at8_sb", [C, 8, 3, C], BF16)
    z16 = sb("z16_sb", [C, 16], F32)
    rz16 = sb("rz16_sb", [C, 16], F32)
    attn = sb("attn_sb", [C, RPC, W], BF16)
    wall = sb("wall_sb", [C, 8, C], BF16)
    ball = sb("ball_sb", [C, 8], F32)
    maskM = sb("maskM_sb", [C, 16, NN], BF16)
    oobc16 = sb("oobc16_sb", [C, 16], F32)
    eshift = sb("eshift_sb", [C, 1], F32)
    yt = sb("yt_sb", [C, 2, CH], BF16)

    W_IDX = {n: k for k, n in enumerate(
        ("wq1t", "wq2t", "wk1t", "wk2t", "wvt", "wat", "wxt", "id"))}
    w_sb = {n: wall[:, k, :] for n, k in W_IDX.items()}
    B_IDX = {n: k for k, n in enumerate(
        ("bq1", "bq2", "bk1", "bk2", "bv", "bo"))}
    b_sb = {n: ball[:, k:k + 1] for n, k in B_IDX.items()}

    ps = nc.alloc_psum_tensor("ps", [C, 8, CH], F32).ap()

    # ---- semaphores / plan ----
    sem_names = ("sde", "sdw", "sp", "sa", "sv", "sg",
                 "sdvt", "sdat", "sdath", "sdout") + tuple(
                     f"sdx{j}" for j in range(6))
    sems = {n: nc.alloc_semaphore(n) for n in sem_names}
    ENGS = ("sync", "pe", "act", "dve", "gp")
    plan = {e: [] for e in ENGS}
    cnt = {n: 0 for n in sem_names}

    def op(eng, fn, sem, inc=1):
        plan[eng].append(("op", fn, sem, inc))
        if sem:
            cnt[sem] += inc
            return (sem, cnt[sem])
        return None

    def wait(eng, mark):
        if mark is not None:
            sem, val = mark
            if val and val > 0:
                plan[eng].append(("w", sem, val))

    # ---- input DMAs (sync ring) ----
    def dma_in(sem, dst, srcd):
        return op("sync", lambda d=dst, s=srcd: nc.sync.dma_start(out=d, in_=s),
                  sem, 16)

    dma_in("sde", wall.rearrange("p a b -> p (a b)"), wall_d)
    SDE = dma_in("sde", ball, ball_d)
    xsp_f = xsp.rearrange("p r w -> p (r w)")
    xs_marks = []
    for i in range(6):
        n = min(CH, NXP - i * CH)
        xs_marks.append(dma_in(f"sdx{i}", xsp_f[:, bass.ds(i * CH, n)],
                               xs_d[:, bass.ds(i * CH, n)]))
    SDW = dma_in("sdw", maskM.rearrange("p a b -> p (a b)"),
                 smask_d.rearrange("p a b -> p (a b)"))
    SDW = dma_in("sdw", oobc16, oobc_d)

    def xdeps(eng, row0, row1):
        """wait for xsp DMA chunks covering padded rows [row0, row1)"""
        c0 = (row0 * WP) // CH
        c1 = (row1 * WP - 1) // CH
        for c in range(c0, c1 + 1):
            wait(eng, xs_marks[c])

    # ---- init memsets (GP) + eshift (DVE) ----
    for c0 in (0, WP - PAD):
        op("gp", lambda tf=kpad[:, :, c0:c0 + PAD]: nc.gpsimd.memset(tf, 0.0),
           "sg")
    op("gp", lambda: nc.gpsimd.memset(vn16[:, :, NN:NN2], 0.0), "sg")
    op("gp", lambda: nc.gpsimd.memset(am8[:, :, NN:NN2], 0.0), "sg")
    MEMSETS = ("sg", cnt["sg"])
    ESHIFT = op("dve", lambda: nc.vector.memset(eshift, EXP_SHIFT), "sv")

    mark = {}      # generic mark table keyed by (stage, idx)

    # =====================================================================
    # PSUM tenancy: pos 0-15 v-blocks, 16-35 conv chunks, 36-51 attn blocks,
    # 52-55 o chunks.  bank = pos % 8.

    def blk_geom(b):
        br, cb = b // 8, b % 8
        return br, cb, 8 * br, cb * 16, b % 8, b % 8   # ..., slot, bank

    # --- v-blocks: per-block conv into vn16 ---
    VEPI_ENG = ["act", "act", "act", "dve", "dve", "dve", "dve", "dve"]

    def emit_vblk(b):
        br, cb, r0, c0, s, _ = blk_geom(b)
        bank = b % 8
        if b % 4 == 0:
            xdeps("pe", r0, r0 + NR)
        if b == 0:
            wait("pe", SDE)
        if b == 8:
            wait("pe", mark[("vepi", 0)])   # banks 0-3 freed (pairs 0,1)
            wait("pe", mark[("vepi", 1)])
        elif b == 12:
            wait("pe", mark[("vepi", 2)])   # banks 4-7 freed (pairs 2,3)
            wait("pe", mark[("vepi", 3)])
        mark[("vmm", b)] = op(
            "pe",
            lambda o=ps[:, bank, 0:NN], l=w_sb["wvt"],
                   r=xsp[:, r0:r0 + NR, c0:c0 + NC_]:
                nc.tensor.matmul(o, l, r, start=True, stop=True),
            "sp")

    def emit_vepi(pr):   # pair pr covers blocks 2pr, 2pr+1
        b0 = 2 * pr
        bank = b0 % 8
        eng = VEPI_ENG[pr]
        wait(eng, SDE)
        wait(eng, mark[("vmm", b0 + 1)])
        wait(eng, MEMSETS)
        fn = (nc.scalar.activation if eng == "act" else None)
        if eng == "act":
            mark[("vepi", pr)] = op(
                "act",
                lambda o=vn16[:, b0:b0 + 2, 0:NN],
                       i_=ps[:, bank:bank + 2, 0:NN], b_=b_sb["bv"]:
                    nc.scalar.activation(o, i_, RELU, bias=b_),
                "sa")
        else:
            mark[("vepi", pr)] = op(
                "dve",
                lambda o=vn16[:, b0:b0 + 2, 0:NN],
                       i_=ps[:, bank:bank + 2, 0:NN], b_=b_sb["bv"]:
                    nc.vector.tensor_scalar(o, i_, b_, 0.0, ADD, MAXOP),
                "sv")

    def emit_vt_quad(g):   # blocks 4g..4g+3 -> vt16, on sync HWDGE ring
        if g == 0:
            wait("sync", MEMSETS)
        for pr in (2 * g, 2 * g + 1):
            wait("sync", mark[("vepi", pr)])
        mark[("vt", g)] = op(
            "sync",
            lambda o=vt16[:, 4 * g:4 * g + 4], i_=vn16[:, 4 * g:4 * g + 4, :]:
                nc.sync.dma_start(out=o, in_=i_, transpose=True),
            "sdvt", 16)

    for b in range(16):
        emit_vblk(b)
        if b % 2 == 1:
            emit_vepi(b // 2)
        if b % 4 == 3:
            emit_vt_quad(b // 4)

    # --- conv chunks ---
    conv_order = (
        [("k1", j) for j in range(4)] + [("q1", 0), ("q1", 1)]
        + [("k2", j) for j in range(4)] + [("q2", 0), ("q2", 1)]
        + [("k1", 4), ("k1", 5)] + [("k2", 4), ("k2", 5)]
        + [("q1", 2), ("q1", 3)] + [("q2", 2), ("q2", 3)]
    )
    epi_groups = [[0, 1], [2, 3], [4, 5], [6, 7], [8, 9], [10, 11],
                  [12], [13], [14], [15], [16, 17], [18, 19]]
    EPI_ENG = {(0, 1): "dve", (2, 3): "dve", (4, 5): "act", (6, 7): "dve",
               (8, 9): "dve", (10, 11): "dve", (12,): "dve", (13,): "dve",
               (14,): "dve", (15,): "dve", (16, 17): "act", (18, 19): "dve"}
    idx_of = {cj: i for i, cj in enumerate(conv_order)}
    epi_of_idx = {}
    for g in epi_groups:
        for i in g:
            epi_of_idx[i] = tuple(g)

    def chunk_cols(cname, j):
        tot = HALO * W if cname in ("k1", "k2") else NPIX
        return min(CH, tot - j * CH)

    CONV_W = {"k1": "wk1t", "q1": "wq1t", "k2": "wk2t", "q2": "wq2t"}
    CONV_B = {"k1": "bk1", "q1": "bq1", "k2": "bk2", "q2": "bq2"}

    def conv_bank(idx):
        # idx 0-11 rotate banks 0-7; tail idx 12-19 packs banks 4-7 so
        # banks 0-3 free early for attention quad 0
        return idx % 8 if idx < 12 else 4 + idx % 4

    def emit_conv_mm(idx):
        cname, j = conv_order[idx]
        n = chunk_cols(cname, j)
        nrows = n // W
        bank = conv_bank(idx)
        if cname == "k1":
            xdeps("pe", 4 * j, 4 * j + nrows)
            rhs = xsp[:, 4 * j:4 * j + nrows, PAD:PAD + W]
        elif cname == "q1":
            xdeps("pe", PAD + 4 * j, PAD + 4 * j + nrows)
            rhs = xsp[:, PAD + 4 * j:PAD + 4 * j + nrows, PAD:PAD + W]
        elif cname == "k2":
            wait("pe", mark[("cepi", epi_of_idx[idx_of[("k1", j)]])])
            rhs = k1[:, bass.ds(j * CH, n)]
        else:
            wait("pe", mark[("cepi", epi_of_idx[idx_of[("q1", j)]])])
            rhs = q1[:, bass.ds(j * CH, n)]
        # bank free
        if idx < 8:
            wait("pe", mark[("vepi", (8 + idx) // 2)])
        elif idx < 16:
            wait("pe", mark[("cepi", epi_of_idx[idx - 8])])
        else:
            wait("pe", mark[("cepi", epi_of_idx[idx - 4])])
        mark[("cmm", idx)] = op(
            "pe",
            lambda o=ps[:, bank, :n], l=w_sb[CONV_W[cname]], r=rhs:
                nc.tensor.matmul(o, l, r, start=True, stop=True),
            "sp")

    def emit_conv_epi(g):
        idx0 = g[0]
        cname, j0 = conv_order[idx0]
        eng = EPI_ENG[tuple(g)]
        nblk = len(g)
        bank0 = conv_bank(idx0)
        wait(eng, SDE)
        wait(eng, mark[("cmm", g[-1])])
        if eng == "dve":
            wait(eng, MEMSETS)
        b_ap = b_sb[CONV_B[cname]]
        ncols = sum(chunk_cols(cname, conv_order[i][1]) for i in g)
        r0 = 4 * j0
        nrows = ncols // W
        src_ap = (ps[:, bank0:bank0 + 2, :] if nblk == 2
                  else ps[:, bank0, :ncols])
        if nblk == 2:
            src_ap = src_ap.rearrange("p a b -> p (a b)")
        if cname == "k1":
            dst = k1[:, bass.ds(j0 * CH, ncols)]
        elif cname == "q1":
            dst = q1[:, bass.ds(j0 * CH, ncols)]
        elif cname == "q2":
            br_ = j0 // 2
            dst = q[:, br_ * 8:(br_ + 1) * 8, :].rearrange(
                "p b (r w) -> p r b w", w=BC)
            src_ap = src_ap.rearrange("p (r b w) -> p r b w", r=8, w=BC)
        else:   # k2 -> kpad interior
            dst = kpad[:, r0:r0 + nrows, PAD:PAD + W]
            src_ap = src_ap.rearrange("p (r w) -> p r w", w=W)
        if eng == "act":
            mark[("cepi", tuple(g))] = op(
                "act",
                lambda o=dst, i_=src_ap, b_=b_ap:
                    nc.scalar.activation(o, i_, RELU, bias=b_),
                "sa")
        else:
            mark[("cepi", tuple(g))] = op(
                "dve",
                lambda o=dst, i_=src_ap, b_=b_ap:
                    nc.vector.tensor_scalar(o, i_, b_, 0.0, ADD, MAXOP),
                "sv")

    gi = 0
    for idx in range(20):
        emit_conv_mm(idx)
        while gi < len(epi_groups) and epi_groups[gi][-1] <= idx:
            emit_conv_epi(epi_groups[gi])
            gi += 1

    # =====================================================================
    # Attention
    def emit_S(b):
        br, cb, r0, c0, s, bank = blk_geom(b)
        mark[("smm", b)] = op(
            "pe",
            lambda o=ps[:, bank, 0:NN], l=q[:, b, :],
                   r=kpad[:, r0:r0 + NR, c0:c0 + NC_]:
                nc.tensor.matmul(o, l, r, start=True, stop=False),
            "sp")
        mark[("mmm", b)] = op(
            "pe",
            lambda o=ps[:, bank, 0:NN], l=w_sb["id"], r=maskM[:, b, :]:
                nc.tensor.matmul(o, l, r, start=False, stop=True),
            "sp")

    def emit_exp(b):
        br, cb, r0, c0, s, bank = blk_geom(b)
        if b == 0:
            wait("act", ESHIFT)
        wait("act", mark[("mmm", b)])
        if b >= 8:
            wait("act", mark[("norm", b - 8)])   # e8 slot free
        mark[("exp", b)] = op(
            "act",
            lambda o=e8[:, s, :], i_=ps[:, bank, 0:NN], sh=eshift,
                   z=z16[:, b:b + 1]:
                nc.scalar.activation(o, i_, EXP, bias=sh, accum_out=z),
            "sa")

    def emit_softmax_quad(g):   # after exp(4g+3): zadd, recip, norms
        b0 = 4 * g
        wait("dve", mark[("exp", b0 + 3)])
        if g == 0:
            wait("dve", SDW)
        zm = op("dve",
                lambda o=z16[:, b0:b0 + 4], i_=z16[:, b0:b0 + 4],
                       i1=oobc16[:, b0:b0 + 4]:
                    nc.vector.tensor_add(o, i_, i1),
                "sv")
        # same-engine RAW: consecutive DVE ops pipeline, so a dependent
        # read needs an explicit sem wait for the writer's completion
        wait("dve", zm)
        rm = op("dve",
                lambda o=rz16[:, b0:b0 + 4], i_=z16[:, b0:b0 + 4]:
                    nc.vector.reciprocal(o, i_),
                "sv")
        wait("dve", rm)
        if g >= 2:
            # am8 slots re-read by the at-DMA halves of quad g-2
            wait("dve", ("sdat", 16 * (g - 1)))
            wait("dve", ("sdath", 16 * (g - 1)))
        for b in range(b0, b0 + 4):
            s = b % 8
            mark[("norm", b)] = op(
                "dve",
                lambda o=am8[:, s, 0:NN], i_=e8[:, s, :],
                       sc=rz16[:, b:b + 1]:
                    nc.vector.tensor_scalar_mul(o, i_, sc),
                "sv")

    def emit_at_quad(g):   # two halves: h0 on sync ring, h1 on ACT ring
        s0 = (4 * g) % 8
        for h, eng, sem in ((0, "sync", "sdat"), (1, "act", "sdath")):
            sh = s0 + 2 * h
            wait(eng, mark[("norm", 4 * g + 2 * h + 1)])
            if g == 0 and h == 0:
                wait("sync", MEMSETS)
            if g >= 2:
                wait(eng, mark[("av", 4 * (g - 2) + 3)])
            mark[("at", g, h)] = op(
                eng,
                lambda o=at8[:, sh:sh + 2], i_=am8[:, sh:sh + 2, :],
                       e_=eng:
                    (nc.sync if e_ == "sync" else nc.scalar).dma_start(
                        out=o, in_=i_, transpose=True),
                sem, 16)

    def emit_av(b):
        br, cb, r0, c0, s, bank = blk_geom(b)
        for ch in range(3):
            mark[("av", b)] = op(
                "pe",
                lambda o=ps[:, bank, NN2:CH], l=vt16[:, b, ch, :],
                       r=at8[:, s, ch, :], st=(ch == 0), sp_=(ch == 2):
                    nc.tensor.matmul(o, l, r, start=st, stop=sp_),
                "sp")

    def emit_acopy(b):   # b even, covers b, b+1
        br, cb, r0, c0, s, bank = blk_geom(b)
        wait("act", mark[("av", b + 1)])
        mark[("acopy", b)] = op(
            "act",
            lambda o=attn[:, r0:r0 + BR, c0:c0 + 2 * BC].rearrange(
                       "p r (a w) -> p a r w", w=BC),
                   i_=ps[:, bank:bank + 2, NN2:CH].rearrange(
                       "p a (r w) -> p a r w", w=BC):
                nc.scalar.copy(o, i_),
            "sa")

    # quad-granular PE batches: waits once per quad, then unbroken MM runs
    # so the PE reorder window can prefetch LDWEIGHTS and pipeline drains.
    def emit_S_quad(g):
        b0 = 4 * g
        br = b0 // 8
        if g == 0:
            wait("pe", SDW)
        wait("pe", mark[("cepi", epi_of_idx[idx_of[("k2", 3 if br == 0 else 5)]
                         ])])
        wait("pe", mark[("cepi", epi_of_idx[idx_of[("q2", 1 if br == 0 else 3)]
                         ])])
        if g == 0:
            for i in (8, 10):      # conv tenants of banks 0-3
                wait("pe", mark[("cepi", epi_of_idx[i])])
        elif g == 1:
            for i in (16, 18):     # conv tail tenants of banks 4-7
                wait("pe", mark[("cepi", epi_of_idx[i])])
        else:
            wait("pe", mark[("acopy", 4 * (g - 2) + 2)])
        for b in range(b0, b0 + 4):
            emit_S(b)
        for b in range(b0, b0 + 4):
            emit_exp(b)
        emit_softmax_quad(g)
        emit_at_quad(g)

    def emit_AV_quad(g):
        wait("pe", ("sdvt", 16 * (g + 1)))
        wait("pe", ("sdat", 16 * (g + 1)))
        wait("pe", ("sdath", 16 * (g + 1)))
        for b in range(4 * g, 4 * g + 4):
            emit_av(b)
        emit_acopy(4 * g)
        emit_acopy(4 * g + 2)

    emit_S_quad(0)
    emit_S_quad(1)
    emit_AV_quad(0)
    emit_S_quad(2)
    emit_AV_quad(1)
    emit_S_quad(3)
    emit_AV_quad(2)
    emit_AV_quad(3)

    # =====================================================================
    # Output conv: banks 4..7 (pos 52-55)
    attn_f = attn.rearrange("p r w -> p (r w)")
    for i in range(4):
        bank = 4 + i
        wait("pe", mark[("acopy", 14 if i >= 2 else 12)])
        op("pe",
           lambda o=ps[:, bank, :], l=w_sb["wat"],
                  r=attn_f[:, bass.ts(i, CH)]:
               nc.tensor.matmul(o, l, r, start=True, stop=False),
           "sp")
        om = op("pe",
                lambda o=ps[:, bank, :], l=w_sb["wxt"],
                       r=xsp[:, PAD + 4 * i:PAD + 4 * i + 4, PAD:PAD + W]:
                    nc.tensor.matmul(o, l, r, start=False, stop=True),
                "sp")
        wait("act", om)
        if i >= 2:
            wait("act", mark[("odma", i - 2)])
        mark[("oepi", i)] = op(
            "act",
            lambda o=yt[:, i % 2, :], i_=ps[:, bank, :], b_=b_sb["bo"]:
                nc.scalar.activation(o, i_, IDENT, bias=b_),
            "sa")
        wait("sync", mark[("oepi", i)])
        mark[("odma", i)] = op(
            "sync",
            lambda o=y_d[:, bass.ts(i, CH)], i_=yt[:, i % 2, :]:
                nc.sync.dma_start(out=o, in_=i_),
            "sdout", 16)

    if DEBUG_OUTS:
        dbg = {
            "d_q": q.rearrange("p a b -> p (a b)"),
            "d_kpad": kpad.rearrange("p r w -> p (r w)"),
            "d_vn": vn16.rearrange("p a b -> p (a b)"),
            "d_vt": vt16.rearrange("p a b c -> p (a b c)"),
            "d_attn": attn.rearrange("p r w -> p (r w)"),
            "d_z": z16,
            "d_rz": rz16,
            "d_am": am8.rearrange("p a b -> p (a b)"),
            "d_at": at8.rearrange("p a b c -> p (a b c)"),
        }
        for nm, src in dbg.items():
            dd = nc.dram_tensor(nm, list(src.shape),
                                src.dtype, kind="ExternalOutput").ap()
            for s_ in ("sp", "sa", "sv"):
                wait("sync", (s_, cnt[s_]))
            op("sync", lambda o=dd, i_=src: nc.sync.dma_start(out=o, in_=i_),
               "sdout", 16)

    # ---- tail barrier ----
    for s_ in ("sp", "sa", "sv", "sg", "sdvt", "sdat", "sdath", "sdout",
               "sde", "sdw"):
        wait("sync", (s_, cnt[s_]))
    for j in range(6):
        wait("sync", (f"sdx{j}", cnt[f"sdx{j}"]))

    # ---- emit ----
    def run(eng_name, eng_obj):
        hwm = {}
        for item in plan[eng_name]:
            if item[0] == "w":
                _, s_, v = item
                if hwm.get(s_, 0) >= v:
                    continue
                hwm[s_] = v
                eng_obj.wait_ge(sems[s_], v)
            else:
                _, fn, s_, inc = item
                inst = fn()
                if s_:
                    inst.then_inc(sems[s_], inc)

    with nc.Block() as block:
        @block.sync
        def _(e):
            run("sync", e)

        @block.tensor
        def _(e):
            run("pe", e)

        @block.scalar
        def _(e):
            run("act", e)

        @block.vector
        def _(e):
            run("dve", e)

        @block.gpsimd
        def _(e):
            run("gp", e)

    with nc.Block() as block2:
        @block2.sync
        def _(e):
            for n in sem_names:
                nc.sync.sem_clear(sems[n])

    return nc


_PROGRAM = None


def _host_inputs(x, w_q1, s_q1, b_q1, w_q2, s_q2, b_q2,
                 w_k1, s_k1, b_k1, w_k2, s_k2, b_k2,
                 w_v, s_v, b_v, w_o, s_o, b_o):
    def foldT(w, s):
        return np.ascontiguousarray((s[:, None] * w).T.astype(ml_dtypes.bfloat16))

    wq1t, wq2t = foldT(w_q1, s_q1), foldT(w_q2, s_q2)
    wk1t, wk2t = foldT(w_k1, s_k1), foldT(w_k2, s_k2)
    wvt = foldT(w_v, s_v)
    wo = s_o[:, None] * w_o
    wat = np.ascontiguousarray(wo[:, :C].T.astype(ml_dtypes.bfloat16))
    wxt = np.ascontiguousarray(wo[:, C:].T.astype(ml_dtypes.bfloat16))

    col = lambda b: np.ascontiguousarray(b.astype(np.float32)[:, None])

    # window-validity over the 14x22 neighborhood, per block pixel
    valid = np.zeros((BR * BC, NR, NC_), bool)
    for r in range(BR):
        for c in range(BC):
            p = r * BC + c
            valid[p, r:r + 7, c:c + 7] = True

    X = np.asarray(x, np.float32).reshape(C, H, W)
    wall = np.concatenate(
        [wq1t, wq2t, wk1t, wk2t, wvt, wat, wxt,
         np.eye(C, dtype=ml_dtypes.bfloat16)], axis=1)
    shared = dict(wall=np.ascontiguousarray(wall))

    e16v = np.float32(np.exp(EXP_SHIFT))
    in_maps = []
    for core in range(NCORES):
        h0 = core * RPC
        xsb = np.zeros((C, HALO, WP), np.float32)
        lo, hi = h0 - PAD, h0 + RPC + PAD
        slo, shi = max(lo, 0), min(hi, H)
        xsb[:, slo - lo:shi - lo, PAD:PAD + W] = X[:, slo:shi]

        # per-block additive mask (0 = in-window & in-image; MASKV else)
        # and oob compensation = (# window positions outside the image)*e^-16.
        # neighborhood row index ri -> image row h0 + br*8 + ri - 3
        # neighborhood col index ci -> image col cb*16 + ci - 3
        maskm = np.empty((16, BR * BC, NN), np.float32)
        oobc = np.empty((16, BR * BC), np.float32)
        for b in range(16):
            brr, cb = b // 8, b % 8
            rowok = np.array([0 <= h0 + brr * BR + ri - PAD < H
                              for ri in range(NR)])
            colok = np.array([0 <= cb * BC + ci - PAD < W
                              for ci in range(NC_)])
            inimg = rowok[:, None] & colok[None, :]
            mb = np.where(valid & inimg[None, :, :], 0.0, MASKV)
            maskm[b] = mb.reshape(BR * BC, NN)
            # per pixel: count of its 49 window positions that are OOB
            n_oob = (valid & ~inimg[None, :, :]).sum(axis=(1, 2))
            oobc[b] = n_oob * e16v
        m = dict(shared)
        m["xs"] = np.ascontiguousarray(
            xsb.reshape(C, NXP).astype(ml_dtypes.bfloat16))
        m["smask"] = np.ascontiguousarray(
            maskm.transpose(1, 0, 2).astype(ml_dtypes.bfloat16))
        m["oobc"] = np.ascontiguousarray(oobc.T.astype(np.float32))
        m["ball"] = np.ascontiguousarray(np.concatenate(
            [col(b_q1), col(b_q2), col(b_k1), col(b_k2), col(b_v),
             col(b_o), np.zeros((C, 2), np.float32)], axis=1))
        in_maps.append(m)
    return in_maps


def kernel(**inputs):
    global _PROGRAM
    if _PROGRAM is None:
        _PROGRAM = _build_program()
    in_maps = _host_inputs(**{k: np.asarray(v) for k, v in inputs.items()})
    res = run_bass_kernel_spmd(_PROGRAM, in_maps, core_ids=list(range(NCORES)))
    stripes = [np.asarray(r["y"]).astype(np.float32).reshape(C, RPC, W)
               for r in res.results]
    return np.concatenate(stripes, axis=1).reshape(1, C, H, W)


if __name__ == "__main__":
    rng = np.random.default_rng(0)
    fake = {"x": rng.standard_normal((1, C, H, W)).astype(np.float32)}
    for n in ("q1", "q2", "k1", "k2", "v", "o"):
        cin = 2 * C if n == "o" else C
        fake["w_" + n] = (rng.standard_normal((C, cin)) / np.sqrt(cin)).astype(np.float32)
        fake["s_" + n] = rng.uniform(0.5, 1.5, C).astype(np.float32)
        fake["b_" + n] = (rng.standard_normal(C) * 0.1).astype(np.float32)
    out = kernel(**fake)
    print("kernel output", out.shape, out.dtype)


# revision 49
# speedup vs baseline: 1.0812x; 1.0812x over previous
"""Trainium2 Bass kernel for ModalitySpecificLocalSelfAttention (7x7 window).

Spatial-parallel over H across 8 cores (16-row stripe + 3-row halo each).
v3 design (from v2's 147us trace; baseline was 98.6us):
  - V path: per-block conv matmuls (moving operand = strided xsp window)
    write each block's 14x22 neighborhood straight into vn16 -- no gather
    copies at all (v2 lost 12us on DVE gathers / 75us on GPSIMD norm).
  - Additive window mask folded into the S accumulation as a second
    matmul (identity stationary); exp then runs with accum_out so the
    softmax denominator z is produced by the ACT pass for free.
    DVE softmax work drops to: batched oob-add + reciprocal + normalize.
  - DMA transposes quad-batched (4 blocks per transfer) and split across
    both HWDGE rings: vt on ACT ring, at on sync ring.
  - 8-bank PSUM tenancy rotation across v-blocks/convs/attention/o-conv.
  - Conv epilogues pair-batched, distributed ACT/DVE; bf16 output DMA.
"""

import sys

for _p in ("/opt/trn_rl_repo", "/root/.axon_site/_ro/trn_rl_repo"):
    if _p not in sys.path:
        sys.path.append(_p)

import ml_dtypes
import numpy as np

import concourse.bass as bass
from concourse import mybir
from concourse.bass_utils import run_bass_kernel_spmd

F32 = mybir.dt.float32
BF16 = mybir.dt.bfloat16

C = 128
H = 128
W = 128
NCORES = 8
RPC = H // NCORES          # 16 rows per core
PAD = 3
HALO = RPC + 2 * PAD       # 22 rows incl halo
WP = W + 2 * PAD           # 134 padded width
BR, BC = 8, 16             # pixel block 8 rows x 16 cols
NR, NC_ = BR + 2 * PAD, BC + 2 * PAD   # 14 x 22 neighborhood
NN = NR * NC_              # 308
NN2 = 384                  # padded to 3x128 for the xbar transpose
NPIX = RPC * W             # 2048
NXP = HALO * WP            # 2948 padded stripe pixels
CH = 512
EXP_SHIFT = -16.0
MASKV = -40.0
DEBUG_OUTS = False

RELU = mybir.ActivationFunctionType.Relu
IDENT = mybir.ActivationFunctionType.Identity
EXP = mybir.ActivationFunctionType.Exp
ADD = mybir.AluOpType.add
MULT = mybir.AluOpType.mult
MAXOP = mybir.AluOpType.max


def _build_program():
    nc = bass.Bass("TRN2", target_bir_lowering=False, debug=False)

    # ---- DRAM I/O ----
    xs_d = nc.dram_tensor("xs", [C, NXP], BF16, kind="ExternalInput").ap()
    wall_d = nc.dram_tensor("wall", [C, 8 * C], BF16, kind="ExternalInput").ap()
    ball_d = nc.dram_tensor("ball", [C, 8], F32, kind="ExternalInput").ap()
    smask_d = nc.dram_tensor("smask", [C, 16, NN], BF16,
                             kind="ExternalInput").ap()
    oobc_d = nc.dram_tensor("oobc", [C, 16], F32, kind="ExternalInput").ap()
    y_d = nc.dram_tensor("y", [C, NPIX], BF16, kind="ExternalOutput").ap()

    # ---- SBUF ----
    sb = lambda name, shape, dt: nc.alloc_sbuf_tensor(name, list(shape), dt).ap()
    xsp = sb("xsp_sb", [C, HALO, WP], BF16)
    k1 = sb("k1_sb", [C, HALO * W], BF16)
    q1 = sb("q1_sb", [C, NPIX], BF16)
    q = sb("q_sb", [C, 16, C], BF16)       # block-major
    kpad = sb("kpad_sb", [C, HALO, WP], BF16)
    vn16 = sb("vn16_sb", [C, 16, NN2], BF16)
    vt16 = sb("vt16_sb", [C, 16, 3, C], BF16)
    e8 = sb("e8_sb", [C, 8, NN], BF16)
    am8 = sb("am8_sb", [C, 8, NN2], BF16)
    at8 = sb("at8_sb", [C, 8, 3, C], BF16)
    z16 = sb("z16_sb", [C, 16], F32)
    rz16 = sb("rz16_sb", [C, 16], F32)
    attn = sb("attn_sb", [C, RPC, W], BF16)
    wall = sb("wall_sb", [C, 8, C], BF16)
    ball = sb("ball_sb", [C, 8], F32)
    maskM = sb("maskM_sb", [C, 16, NN], BF16)
    oobc16 = sb("oobc16_sb", [C, 16], F32)
    eshift = sb("eshift_sb", [C, 1], F32)
    yt = sb("yt_sb", [C, 2, CH], BF16)

    W_IDX = {n: k for k, n in enumerate(
        ("wq1t", "wq2t", "wk1t", "wk2t", "wvt", "wat", "wxt", "id"))}
    w_sb = {n: wall[:, k, :] for n, k in W_IDX.items()}
    B_IDX = {n: k for k, n in enumerate(
        ("bq1", "bq2", "bk1", "bk2", "bv", "bo"))}
    b_sb = {n: ball[:, k:k + 1] for n, k in B_IDX.items()}

    ps = nc.alloc_psum_tensor("ps", [C, 8, CH], F32).ap()

    # ---- semaphores / plan ----
    sem_names = ("sde", "sdw", "sp", "sa", "sv", "sg",
                 "sdvt", "sdat", "sdath", "sdout") + tuple(
                     f"sdx{j}" for j in range(6))
    sems = {n: nc.alloc_semaphore(n) for n in sem_names}
    ENGS = ("sync", "pe", "act", "dve", "gp")
    plan = {e: [] for e in ENGS}
    cnt = {n: 0 for n in sem_names}

    def op(eng, fn, sem, inc=1):
        plan[eng].append(("op", fn, sem, inc))
        if sem:
            cnt[sem] += inc
            return (sem, cnt[sem])
        return None

    def wait(eng, mark):
        if mark is not None:
            sem, val = mark
            if val and val > 0:
                plan[eng].append(("w", sem, val))

    # ---- input DMAs (sync ring) ----
    def dma_in(sem, dst, srcd):
        return op("sync", lambda d=dst, s=srcd: nc.sync.dma_start(out=d, in_=s),
                  sem, 16)

    dma_in("sde", wall.rearrange("p a b -> p (a b)"), wall_d)
    SDE = dma_in("sde", ball, ball_d)
    xsp_f = xsp.rearrange("p r w -> p (r w)")
    xs_marks = []
    for i in range(6):
        n = min(CH, NXP - i * CH)
        xs_marks.append(dma_in(f"sdx{i}", xsp_f[:, bass.ds(i * CH, n)],
                               xs_d[:, bass.ds(i * CH, n)]))
    SDW = dma_in("sdw", maskM.rearrange("p a b -> p (a b)"),
                 smask_d.rearrange("p a b -> p (a b)"))
    SDW = dma_in("sdw", oobc16, oobc_d)

    def xdeps(eng, row0, row1):
        """wait for xsp DMA chunks covering padded rows [row0, row1)"""
        c0 = (row0 * WP) // CH
        c1 = (row1 * WP - 1) // CH
        for c in range(c0, c1 + 1):
            wait(eng, xs_marks[c])

    # ---- init memsets (GP) + eshift (DVE) ----
    for c0 in (0, WP - PAD):
        op("gp", lambda tf=kpad[:, :, c0:c0 + PAD]: nc.gpsimd.memset(tf, 0.0),
           "sg")
    op("gp", lambda: nc.gpsimd.memset(vn16[:, :, NN:NN2], 0.0), "sg")
    op("gp", lambda: nc.gpsimd.memset(am8[:, :, NN:NN2], 0.0), "sg")
    MEMSETS = ("sg", cnt["sg"])
    ESHIFT = op("dve", lambda: nc.vector.memset(eshift, EXP_SHIFT), "sv")

    mark = {}      # generic mark table keyed by (stage, idx)

    # =====================================================================
    # PSUM tenancy: pos 0-15 v-blocks, 16-35 conv chunks, 36-51 attn blocks,
    # 52-55 o chunks.  bank = pos % 8.

    def blk_geom(b):
        br, cb = b // 8, b % 8
        return br, cb, 8 * br, cb * 16, b % 8, b % 8   # ..., slot, bank

    # --- v-blocks: per-block conv into vn16 ---
    VEPI_ENG = ["act", "act", "act", "dve", "dve", "dve", "dve", "dve"]

    def emit_vblk(b):
        br, cb, r0, c0, s, _ = blk_geom(b)
        bank = b % 8
        if b % 4 == 0:
            xdeps("pe", r0, r0 + NR)
        if b == 0:
            wait("pe", SDE)
        if b == 8:
            wait("pe", mark[("vepi", 0)])   # banks 0-3 freed (pairs 0,1)
            wait("pe", mark[("vepi", 1)])
        elif b == 12:
            wait("pe", mark[("vepi", 2)])   # banks 4-7 freed (pairs 2,3)
            wait("pe", mark[("vepi", 3)])
        mark[("vmm", b)] = op(
            "pe",
            lambda o=ps[:, bank, 0:NN], l=w_sb["wvt"],
                   r=xsp[:, r0:r0 + NR, c0:c0 + NC_]:
                nc.tensor.matmul(o, l, r, start=True, stop=True),
            "sp")

    def emit_vepi(pr):   # pair pr covers blocks 2pr, 2pr+1
        b0 = 2 * pr
        bank = b0 % 8
        eng = VEPI_ENG[pr]
        wait(eng, SDE)
        wait(eng, mark[("vmm", b0 + 1)])
        wait(eng, MEMSETS)
        fn = (nc.scalar.activation if eng == "act" else None)
        if eng == "act":
            mark[("vepi", pr)] = op(
                "act",
                lambda o=vn16[:, b0:b0 + 2, 0:NN],
                       i_=ps[:, bank:bank + 2, 0:NN], b_=b_sb["bv"]:
                    nc.scalar.activation(o, i_, RELU, bias=b_),
                "sa")
        else:
            mark[("vepi", pr)] = op(
                "dve",
                lambda o=vn16[:, b0:b0 + 2, 0:NN],
                       i_=ps[:, bank:bank + 2, 0:NN], b_=b_sb["bv"]:
                    nc.vector.tensor_scalar(o, i_, b_, 0.0, ADD, MAXOP),
                "sv")

    def emit_vt_quad(g):   # blocks 4g..4g+3 -> vt16, on sync HWDGE ring
        if g == 0:
            wait("sync", MEMSETS)
        for pr in (2 * g, 2 * g + 1):
            wait("sync", mark[("vepi", pr)])
        mark[("vt", g)] = op(
            "sync",
            lambda o=vt16[:, 4 * g:4 * g + 4], i_=vn16[:, 4 * g:4 * g + 4, :]:
                nc.sync.dma_start(out=o, in_=i_, transpose=True),
            "sdvt", 16)

    for b in range(16):
        emit_vblk(b)
        if b % 2 == 1:
            emit_vepi(b // 2)
        if b % 4 == 3:
            emit_vt_quad(b // 4)

    # --- conv chunks ---
    conv_order = (
        [("k1", j) for j in range(4)] + [("q1", 0), ("q1", 1)]
        + [("k2", j) for j in range(4)] + [("q2", 0), ("q2", 1)]
        + [("k1", 4), ("k1", 5)] + [("k2", 4), ("k2", 5)]
        + [("q1", 2), ("q1", 3)] + [("q2", 2), ("q2", 3)]
    )
    epi_groups = [[0, 1], [2, 3], [4, 5], [6, 7], [8, 9], [10, 11],
                  [12], [13], [14], [15], [16, 17], [18, 19]]
    EPI_ENG = {(0, 1): "dve", (2, 3): "dve", (4, 5): "act", (6, 7): "dve",
               (8, 9): "dve", (10, 11): "dve", (12,): "dve", (13,): "dve",
               (14,): "dve", (15,): "dve", (16, 17): "act", (18, 19): "dve"}
    idx_of = {cj: i for i, cj in enumerate(conv_order)}
    epi_of_idx = {}
    for g in epi_groups:
        for i in g:
            epi_of_idx[i] = tuple(g)

    def chunk_cols(cname, j):
        tot = HALO * W if cname in ("k1", "k2") else NPIX
        return min(CH, tot - j * CH)

    CONV_W = {"k1": "wk1t", "q1": "wq1t", "k2": "wk2t", "q2": "wq2t"}
    CONV_B = {"k1": "bk1", "q1": "bq1", "k2": "bk2", "q2": "bq2"}

    def conv_bank(idx):
        # idx 0-11 rotate banks 0-7; tail idx 12-19 packs banks 4-7 so
        # banks 0-3 free early for attention quad 0
        return idx % 8 if idx < 12 else 4 + idx % 4

    def emit_conv_mm(idx):
        cname, j = conv_order[idx]
        n = chunk_cols(cname, j)
        nrows = n // W
        bank = conv_bank(idx)
        if cname == "k1":
            xdeps("pe", 4 * j, 4 * j + nrows)
            rhs = xsp[:, 4 * j:4 * j + nrows, PAD:PAD + W]
        elif cname == "q1":
            xdeps("pe", PAD + 4 * j, PAD + 4 * j + nrows)
            rhs = xsp[:, PAD + 4 * j:PAD + 4 * j + nrows, PAD:PAD + W]
        elif cname == "k2":
            wait("pe", mark[("cepi", epi_of_idx[idx_of[("k1", j)]])])
            rhs = k1[:, bass.ds(j * CH, n)]
        else:
            wait("pe", mark[("cepi", epi_of_idx[idx_of[("q1", j)]])])
            rhs = q1[:, bass.ds(j * CH, n)]
        # bank free
        if idx < 8:
            wait("pe", mark[("vepi", (8 + idx) // 2)])
        elif idx < 16:
            wait("pe", mark[("cepi", epi_of_idx[idx - 8])])
        else:
            wait("pe", mark[("cepi", epi_of_idx[idx - 4])])
        mark[("cmm", idx)] = op(
            "pe",
            lambda o=ps[:, bank, :n], l=w_sb[CONV_W[cname]], r=rhs:
                nc.tensor.matmul(o, l, r, start=True, stop=True),
            "sp")

    def emit_conv_epi(g):
        idx0 = g[0]
        cname, j0 = conv_order[idx0]
        eng = EPI_ENG[tuple(g)]
        nblk = len(g)
        bank0 = conv_bank(idx0)
        wait(eng, SDE)
        wait(eng, mark[("cmm", g[-1])])
        if eng == "dve":
            wait(eng, MEMSETS)
        b_ap = b_sb[CONV_B[cname]]
        ncols = sum(chunk_cols(cname, conv_order[i][1]) for i in g)
        r0 = 4 * j0
        nrows = ncols // W
        src_ap = (ps[:, bank0:bank0 + 2, :] if nblk == 2
                  else ps[:, bank0, :ncols])
        if nblk == 2:
            src_ap = src_ap.rearrange("p a b -> p (a b)")
        if cname == "k1":
            dst = k1[:, bass.ds(j0 * CH, ncols)]
        elif cname == "q1":
            dst = q1[:, bass.ds(j0 * CH, ncols)]
        elif cname == "q2":
            br_ = j0 // 2
            dst = q[:, br_ * 8:(br_ + 1) * 8, :].rearrange(
                "p b (r w) -> p r b w", w=BC)
            src_ap = src_ap.rearrange("p (r b w) -> p r b w", r=8, w=BC)
        else:   # k2 -> kpad interior
            dst = kpad[:, r0:r0 + nrows, PAD:PAD + W]
            src_ap = src_ap.rearrange("p (r w) -> p r w", w=W)
        if eng == "act":
            mark[("cepi", tuple(g))] = op(
                "act",
                lambda o=dst, i_=src_ap, b_=b_ap:
                    nc.scalar.activation(o, i_, RELU, bias=b_),
                "sa")
        else:
            mark[("cepi", tuple(g))] = op(
                "dve",
                lambda o=dst, i_=src_ap, b_=b_ap:
                    nc.vector.tensor_scalar(o, i_, b_, 0.0, ADD, MAXOP),
                "sv")

    gi = 0
    for idx in range(20):
        emit_conv_mm(idx)
        while gi < len(epi_groups) and epi_groups[gi][-1] <= idx:
            emit_conv_epi(epi_groups[gi])
            gi += 1

    # =====================================================================
    # Attention
    def emit_S(b):
        br, cb, r0, c0, s, bank = blk_geom(b)
        mark[("smm", b)] = op(
            "pe",
            lambda o=ps[:, bank, 0:NN], l=q[:, b, :],
                   r=kpad[:, r0:r0 + NR, c0:c0 + NC_]:
                nc.tensor.matmul(o, l, r, start=True, stop=False),
            "sp")
        mark[("mmm", b)] = op(
            "pe",
            lambda o=ps[:, bank, 0:NN], l=w_sb["id"], r=maskM[:, b, :]:
                nc.tensor.matmul(o, l, r, start=False, stop=True),
            "sp")

    def emit_exp_pair(b0):   # blocks b0, b0+1 in one ACT op (no accum)
        s0 = b0 % 8
        bank0 = b0 % 8
        if b0 == 0:
            wait("act", ESHIFT)
        wait("act", mark[("mmm", b0 + 1)])
        if b0 >= 8:
            wait("act", mark[("norm", b0 - 7)])   # e8 slots free
        mark[("exp", b0)] = op(
            "act",
            lambda o=e8[:, s0:s0 + 2, :], i_=ps[:, bank0:bank0 + 2, 0:NN],
                   sh=eshift:
                nc.scalar.activation(o, i_, EXP, bias=sh),
            "sa")

    def emit_softmax_pair(b0):   # z reduce + oob + recip + norms x2
        s0 = b0 % 8
        g = b0 // 4
        pr = (b0 % 4) // 2
        wait("dve", mark[("exp", b0)])
        if b0 == 0:
            wait("dve", SDW)
        zm = op("dve",
                lambda o=z16[:, b0:b0 + 2], i_=e8[:, s0:s0 + 2, :]:
                    nc.vector.reduce_sum(o, i_, axis=mybir.AxisListType.X),
                "sv")
        # same-engine RAW: consecutive DVE ops pipeline; dependent reads
        # need an explicit sem wait for the writer's completion
        wait("dve", zm)
        zm = op("dve",
                lambda o=z16[:, b0:b0 + 2], i_=z16[:, b0:b0 + 2],
                       i1=oobc16[:, b0:b0 + 2]:
                    nc.vector.tensor_add(o, i_, i1),
                "sv")
        wait("dve", zm)
        rm = op("dve",
                lambda o=rz16[:, b0:b0 + 2], i_=z16[:, b0:b0 + 2]:
                    nc.vector.reciprocal(o, i_),
                "sv")
        wait("dve", rm)
        if g >= 2:
            # am8 slots re-read by the at-half of quad g-2, pair pr
            wait("dve", ("sdat", 16 * (2 * (g - 2) + pr + 1)))
        for b in (b0, b0 + 1):
            s = b % 8
            mark[("norm", b)] = op(
                "dve",
                lambda o=am8[:, s, 0:NN], i_=e8[:, s, :],
                       sc=rz16[:, b:b + 1]:
                    nc.vector.tensor_scalar_mul(o, i_, sc),
                "sv")

    def emit_at_half(b0):   # 2 blocks per transpose, sync ring
        s0 = b0 % 8
        g = b0 // 4
        wait("sync", mark[("norm", b0 + 1)])
        if b0 == 0:
            wait("sync", MEMSETS)
        if g >= 2:
            wait("sync", mark[("av", b0 - 7)])
        mark[("at", b0)] = op(
            "sync",
            lambda o=at8[:, s0:s0 + 2], i_=am8[:, s0:s0 + 2, :]:
                nc.sync.dma_start(out=o, in_=i_, transpose=True),
            "sdat", 16)

    def emit_av(b):
        br, cb, r0, c0, s, bank = blk_geom(b)
        for ch in range(3):
            mark[("av", b)] = op(
                "pe",
                lambda o=ps[:, bank, NN2:CH], l=vt16[:, b, ch, :],
                       r=at8[:, s, ch, :], st=(ch == 0), sp_=(ch == 2):
                    nc.tensor.matmul(o, l, r, start=st, stop=sp_),
                "sp")

    def emit_acopy(b):   # b even, covers b, b+1
        br, cb, r0, c0, s, bank = blk_geom(b)
        wait("act", mark[("av", b + 1)])
        mark[("acopy", b)] = op(
            "act",
            lambda o=attn[:, r0:r0 + BR, c0:c0 + 2 * BC].rearrange(
                       "p r (a w) -> p a r w", w=BC),
                   i_=ps[:, bank:bank + 2, NN2:CH].rearrange(
                       "p a (r w) -> p a r w", w=BC):
                nc.scalar.copy(o, i_),
            "sa")

    # quad-granular PE batches: waits once per quad, then unbroken MM runs
    # so the PE reorder window can prefetch LDWEIGHTS and pipeline drains.
    # Softmax/transpose run per PAIR of blocks to shorten the chain that
    # gates each AV pair.
    def emit_S_quad(g):
        b0 = 4 * g
        br = b0 // 8
        if g == 0:
            wait("pe", SDW)
        wait("pe", mark[("cepi", epi_of_idx[idx_of[("k2", 3 if br == 0 else 5)]
                         ])])
        wait("pe", mark[("cepi", epi_of_idx[idx_of[("q2", 1 if br == 0 else 3)]
                         ])])
        if g == 0:
            for i in (8, 10):      # conv tenants of banks 0-3
                wait("pe", mark[("cepi", epi_of_idx[i])])
        elif g == 1:
            for i in (16, 18):     # conv tail tenants of banks 4-7
                wait("pe", mark[("cepi", epi_of_idx[i])])
        else:
            wait("pe", mark[("acopy", 4 * (g - 2) + 2)])
        for b in range(b0, b0 + 4):
            emit_S(b)
        for p in (0, 2):
            emit_exp_pair(b0 + p)
            emit_softmax_pair(b0 + p)
            emit_at_half(b0 + p)

    def emit_AV_pair(b0):
        g, pr = b0 // 4, (b0 % 4) // 2
        wait("pe", ("sdvt", 16 * (g + 1)))
        wait("pe", ("sdat", 16 * (2 * g + pr + 1)))
        emit_av(b0)
        emit_av(b0 + 1)
        emit_acopy(b0)

    emit_S_quad(0)
    emit_S_quad(1)
    emit_AV_pair(0)
    emit_AV_pair(2)
    emit_S_quad(2)
    emit_AV_pair(4)
    emit_AV_pair(6)
    emit_S_quad(3)
    emit_AV_pair(8)
    emit_AV_pair(10)
    emit_AV_pair(12)
    emit_AV_pair(14)

    # =====================================================================
    # Output conv: banks 4..7 (pos 52-55)
    attn_f = attn.rearrange("p r w -> p (r w)")
    for i in range(4):
        bank = 4 + i
        wait("pe", mark[("acopy", 14 if i >= 2 else 12)])
        op("pe",
           lambda o=ps[:, bank, :], l=w_sb["wat"],
                  r=attn_f[:, bass.ts(i, CH)]:
               nc.tensor.matmul(o, l, r, start=True, stop=False),
           "sp")
        om = op("pe",
                lambda o=ps[:, bank, :], l=w_sb["wxt"],
                       r=xsp[:, PAD + 4 * i:PAD + 4 * i + 4, PAD:PAD + W]:
                    nc.tensor.matmul(o, l, r, start=False, stop=True),
                "sp")
        wait("act", om)
        if i >= 2:
            wait("act", mark[("odma", i - 2)])
        mark[("oepi", i)] = op(
            "act",
            lambda o=yt[:, i % 2, :], i_=ps[:, bank, :], b_=b_sb["bo"]:
                nc.scalar.activation(o, i_, IDENT, bias=b_),
            "sa")
        wait("sync", mark[("oepi", i)])
        mark[("odma", i)] = op(
            "sync",
            lambda o=y_d[:, bass.ts(i, CH)], i_=yt[:, i % 2, :]:
                nc.sync.dma_start(out=o, in_=i_),
            "sdout", 16)

    if DEBUG_OUTS:
        dbg = {
            "d_q": q.rearrange("p a b -> p (a b)"),
            "d_kpad": kpad.rearrange("p r w -> p (r w)"),
            "d_vn": vn16.rearrange("p a b -> p (a b)"),
            "d_vt": vt16.rearrange("p a b c -> p (a b c)"),
            "d_attn": attn.rearrange("p r w -> p (r w)"),
            "d_z": z16,
            "d_rz": rz16,
            "d_am": am8.rearrange("p a b -> p (a b)"),
            "d_at": at8.rearrange("p a b c -> p (a b c)"),
        }
        for nm, src in dbg.items():
            dd = nc.dram_tensor(nm, list(src.shape),
                                src.dtype, kind="ExternalOutput").ap()
            for s_ in ("sp", "sa", "sv"):
                wait("sync", (s_, cnt[s_]))
            op("sync", lambda o=dd, i_=src: nc.sync.dma_start(out=o, in_=i_),
               "sdout", 16)

    # ---- tail barrier ----
    for s_ in ("sp", "sa", "sv", "sg", "sdvt", "sdat", "sdath", "sdout",
               "sde", "sdw"):
        wait("sync", (s_, cnt[s_]))
    for j in range(6):
        wait("sync", (f"sdx{j}", cnt[f"sdx{j}"]))

    # ---- emit ----
    def run(eng_name, eng_obj):
        hwm = {}
        for item in plan[eng_name]:
            if item[0] == "w":
                _, s_, v = item
                if hwm.get(s_, 0) >= v:
                    continue
                hwm[s_] = v
                eng_obj.wait_ge(sems[s_], v)
            else:
                _, fn, s_, inc = item
                inst = fn()
                if s_:
                    inst.then_inc(sems[s_], inc)

    with nc.Block() as block:
        @block.sync
        def _(e):
            run("sync", e)

        @block.tensor
        def _(e):
            run("pe", e)

        @block.scalar
        def _(e):
            run("act", e)

        @block.vector
        def _(e):
            run("dve", e)

        @block.gpsimd
        def _(e):
            run("gp", e)

    with nc.Block() as block2:
        @block2.sync
        def _(e):
            for n in sem_names:
                nc.sync.sem_clear(sems[n])

    return nc


_PROGRAM = None


def _host_inputs(x, w_q1, s_q1, b_q1, w_q2, s_q2, b_q2,
                 w_k1, s_k1, b_k1, w_k2, s_k2, b_k2,
                 w_v, s_v, b_v, w_o, s_o, b_o):
    def foldT(w, s):
        return np.ascontiguousarray((s[:, None] * w).T.astype(ml_dtypes.bfloat16))

    wq1t, wq2t = foldT(w_q1, s_q1), foldT(w_q2, s_q2)
    wk1t, wk2t = foldT(w_k1, s_k1), foldT(w_k2, s_k2)
    wvt = foldT(w_v, s_v)
    wo = s_o[:, None] * w_o
    wat = np.ascontiguousarray(wo[:, :C].T.astype(ml_dtypes.bfloat16))
    wxt = np.ascontiguousarray(wo[:, C:].T.astype(ml_dtypes.bfloat16))

    col = lambda b: np.ascontiguousarray(b.astype(np.float32)[:, None])

    # window-validity over the 14x22 neighborhood, per block pixel
    valid = np.zeros((BR * BC, NR, NC_), bool)
    for r in range(BR):
        for c in range(BC):
            p = r * BC + c
            valid[p, r:r + 7, c:c + 7] = True

    X = np.asarray(x, np.float32).reshape(C, H, W)
    wall = np.concatenate(
        [wq1t, wq2t, wk1t, wk2t, wvt, wat, wxt,
         np.eye(C, dtype=ml_dtypes.bfloat16)], axis=1)
    shared = dict(wall=np.ascontiguousarray(wall))

    e16v = np.float32(np.exp(EXP_SHIFT))
    in_maps = []
    for core in range(NCORES):
        h0 = core * RPC
        xsb = np.zeros((C, HALO, WP), np.float32)
        lo, hi = h0 - PAD, h0 + RPC + PAD
        slo, shi = max(lo, 0), min(hi, H)
        xsb[:, slo - lo:shi - lo, PAD:PAD + W] = X[:, slo:shi]

        # per-block additive mask (0 = in-window & in-image; MASKV else)
        # and oob compensation = (# window positions outside the image)*e^-16.
        # neighborhood row index ri -> image row h0 + br*8 + ri - 3
        # neighborhood col index ci -> image col cb*16 + ci - 3
        maskm = np.empty((16, BR * BC, NN), np.float32)
        oobc = np.empty((16, BR * BC), np.float32)
        for b in range(16):
            brr, cb = b // 8, b % 8
            rowok = np.array([0 <= h0 + brr * BR + ri - PAD < H
                              for ri in range(NR)])
            colok = np.array([0 <= cb * BC + ci - PAD < W
                              for ci in range(NC_)])
            inimg = rowok[:, None] & colok[None, :]
            mb = np.where(valid & inimg[None, :, :], 0.0, MASKV)
            maskm[b] = mb.reshape(BR * BC, NN)
            # per pixel: count of its 49 window positions that are OOB
            n_oob = (valid & ~inimg[None, :, :]).sum(axis=(1, 2))
            oobc[b] = n_oob * e16v
        m = dict(shared)
        m["xs"] = np.ascontiguousarray(
            xsb.reshape(C, NXP).astype(ml_dtypes.bfloat16))
        m["smask"] = np.ascontiguousarray(
            maskm.transpose(1, 0, 2).astype(ml_dtypes.bfloat16))
        m["oobc"] = np.ascontiguousarray(oobc.T.astype(np.float32))
        m["ball"] = np.ascontiguousarray(np.concatenate(
            [col(b_q1), col(b_q2), col(b_k1), col(b_k2), col(b_v),
             col(b_o), np.zeros((C, 2), np.float32)], axis=1))
        in_maps.append(m)
    return in_maps


def kernel(**inputs):
    global _PROGRAM
    if _PROGRAM is None:
        _PROGRAM = _build_program()
    in_maps = _host_inputs(**{k: np.asarray(v) for k, v in inputs.items()})
    res = run_bass_kernel_spmd(_PROGRAM, in_maps, core_ids=list(range(NCORES)))
    stripes = [np.asarray(r["y"]).astype(np.float32).reshape(C, RPC, W)
               for r in res.results]
    return np.concatenate(stripes, axis=1).reshape(1, C, H, W)


if __name__ == "__main__":
    rng = np.random.default_rng(0)
    fake = {"x": rng.standard_normal((1, C, H, W)).astype(np.float32)}
    for n in ("q1", "q2", "k1", "k2", "v", "o"):
        cin = 2 * C if n == "o" else C
        fake["w_" + n] = (rng.standard_normal((C, cin)) / np.sqrt(cin)).astype(np.float32)
        fake["s_" + n] = rng.uniform(0.5, 1.5, C).astype(np.float32)
        fake["b_" + n] = (rng.standard_normal(C) * 0.1).astype(np.float32)
    out = kernel(**fake)
    print("kernel output", out.shape, out.dtype)


# revision 55
# speedup vs baseline: 1.1909x; 1.1014x over previous
"""Trainium2 Bass kernel for ModalitySpecificLocalSelfAttention (7x7 window).

Spatial-parallel over H across 8 cores (16-row stripe + 3-row halo each).
v3 design (from v2's 147us trace; baseline was 98.6us):
  - V path: per-block conv matmuls (moving operand = strided xsp window)
    write each block's 14x22 neighborhood straight into vn16 -- no gather
    copies at all (v2 lost 12us on DVE gathers / 75us on GPSIMD norm).
  - Additive window mask folded into the S accumulation as a second
    matmul (identity stationary); exp then runs with accum_out so the
    softmax denominator z is produced by the ACT pass for free.
    DVE softmax work drops to: batched oob-add + reciprocal + normalize.
  - DMA transposes quad-batched (4 blocks per transfer) and split across
    both HWDGE rings: vt on ACT ring, at on sync ring.
  - 8-bank PSUM tenancy rotation across v-blocks/convs/attention/o-conv.
  - Conv epilogues pair-batched, distributed ACT/DVE; bf16 output DMA.
"""

import sys

for _p in ("/opt/trn_rl_repo", "/root/.axon_site/_ro/trn_rl_repo"):
    if _p not in sys.path:
        sys.path.append(_p)

import ml_dtypes
import numpy as np

import concourse.bass as bass
from concourse import mybir
from concourse.bass_utils import run_bass_kernel_spmd

F32 = mybir.dt.float32
BF16 = mybir.dt.bfloat16

C = 128
H = 128
W = 128
NCORES = 8
RPC = H // NCORES          # 16 rows per core
PAD = 3
HALO = RPC + 2 * PAD       # 22 rows incl halo
WP = W + 2 * PAD           # 134 padded width
BR, BC = 8, 16             # pixel block 8 rows x 16 cols
NR, NC_ = BR + 2 * PAD, BC + 2 * PAD   # 14 x 22 neighborhood
NN = NR * NC_              # 308
NN2 = 384                  # padded to 3x128 for the xbar transpose
NPIX = RPC * W             # 2048
NXP = HALO * WP            # 2948 padded stripe pixels
CH = 512
EXP_SHIFT = -16.0
MASKV = -40.0
DEBUG_OUTS = False

RELU = mybir.ActivationFunctionType.Relu
IDENT = mybir.ActivationFunctionType.Identity
EXP = mybir.ActivationFunctionType.Exp
ADD = mybir.AluOpType.add
MULT = mybir.AluOpType.mult
MAXOP = mybir.AluOpType.max


def _build_program():
    nc = bass.Bass("TRN2", target_bir_lowering=False, debug=False)

    # ---- DRAM I/O ----
    xs_d = nc.dram_tensor("xs", [C, NXP], BF16, kind="ExternalInput").ap()
    wall_d = nc.dram_tensor("wall", [C, 8 * C], BF16, kind="ExternalInput").ap()
    ball_d = nc.dram_tensor("ball", [C, 8], F32, kind="ExternalInput").ap()
    smask_d = nc.dram_tensor("smask", [C, 16, NN], BF16,
                             kind="ExternalInput").ap()
    oobc_d = nc.dram_tensor("oobc", [C, 16], F32, kind="ExternalInput").ap()
    y_d = nc.dram_tensor("y", [C, NPIX], BF16, kind="ExternalOutput").ap()

    # ---- SBUF ----
    sb = lambda name, shape, dt: nc.alloc_sbuf_tensor(name, list(shape), dt).ap()
    xsp = sb("xsp_sb", [C, HALO, WP], BF16)
    k1 = sb("k1_sb", [C, HALO * W], BF16)
    q1 = sb("q1_sb", [C, NPIX], BF16)
    q = sb("q_sb", [C, 16, C], BF16)       # block-major
    kpad = sb("kpad_sb", [C, HALO, WP], BF16)
    vn16 = sb("vn16_sb", [C, 16, NN2], BF16)
    vt16 = sb("vt16_sb", [C, 16, 3, C], BF16)
    e8 = sb("e8_sb", [C, 8, NN], BF16)
    am8 = sb("am8_sb", [C, 8, NN2], BF16)
    at8 = sb("at8_sb", [C, 8, 3, C], BF16)
    z16 = sb("z16_sb", [C, 16], F32)
    rz16 = sb("rz16_sb", [C, 16], F32)
    attn = sb("attn_sb", [C, RPC, W], BF16)
    wall = sb("wall_sb", [C, 8, C], BF16)
    ball = sb("ball_sb", [C, 8], F32)
    maskM = sb("maskM_sb", [C, 16, NN], BF16)
    oobc16 = sb("oobc16_sb", [C, 16], F32)
    eshift = sb("eshift_sb", [C, 1], F32)
    yt = sb("yt_sb", [C, 2, CH], BF16)

    W_IDX = {n: k for k, n in enumerate(
        ("wq1t", "wq2t", "wk1t", "wk2t", "wvt", "wat", "wxt", "id"))}
    w_sb = {n: wall[:, k, :] for n, k in W_IDX.items()}
    B_IDX = {n: k for k, n in enumerate(
        ("bq1", "bq2", "bk1", "bk2", "bv", "bo"))}
    b_sb = {n: ball[:, k:k + 1] for n, k in B_IDX.items()}

    ps = nc.alloc_psum_tensor("ps", [C, 8, CH], F32).ap()

    # ---- semaphores / plan ----
    sem_names = ("sde", "sdw", "sp", "sa", "sv", "sg",
                 "sdvt", "sdat", "sdath", "sdout") + tuple(
                     f"sdx{j}" for j in range(6))
    sems = {n: nc.alloc_semaphore(n) for n in sem_names}
    ENGS = ("sync", "pe", "act", "dve", "gp")
    plan = {e: [] for e in ENGS}
    cnt = {n: 0 for n in sem_names}

    def op(eng, fn, sem, inc=1):
        plan[eng].append(("op", fn, sem, inc))
        if sem:
            cnt[sem] += inc
            return (sem, cnt[sem])
        return None

    def wait(eng, mark):
        if mark is not None:
            sem, val = mark
            if val and val > 0:
                plan[eng].append(("w", sem, val))

    # ---- input DMAs (sync ring) ----
    def dma_in(sem, dst, srcd):
        return op("sync", lambda d=dst, s=srcd: nc.sync.dma_start(out=d, in_=s),
                  sem, 16)

    def dma_in2(eng, sem, dst, srcd):
        fn = nc.sync.dma_start if eng == "sync" else nc.scalar.dma_start
        return op(eng, lambda d=dst, s=srcd, f=fn: f(out=d, in_=s), sem, 16)

    # inputs split across both HWDGE rings
    dma_in2("sync", "sde", wall.rearrange("p a b -> p (a b)"), wall_d)
    SDE = dma_in2("act", "sde", ball, ball_d)
    xsp_f = xsp.rearrange("p r w -> p (r w)")
    xs_marks = []
    for i in range(6):
        n = min(CH, NXP - i * CH)
        eng = "sync" if i % 2 == 0 else "act"
        xs_marks.append(dma_in2(eng, f"sdx{i}", xsp_f[:, bass.ds(i * CH, n)],
                                xs_d[:, bass.ds(i * CH, n)]))
    dma_in2("act", "sdw", maskM.rearrange("p a b -> p (a b)"),
            smask_d.rearrange("p a b -> p (a b)"))
    SDW = dma_in2("act", "sdw", oobc16, oobc_d)

    def xdeps(eng, row0, row1):
        """wait for xsp DMA chunks covering padded rows [row0, row1)"""
        c0 = (row0 * WP) // CH
        c1 = (row1 * WP - 1) // CH
        for c in range(c0, c1 + 1):
            wait(eng, xs_marks[c])

    # ---- init memsets (GP) + eshift (DVE) ----
    for c0 in (0, WP - PAD):
        op("gp", lambda tf=kpad[:, :, c0:c0 + PAD]: nc.gpsimd.memset(tf, 0.0),
           "sg")
    op("gp", lambda: nc.gpsimd.memset(vn16[:, :, NN:NN2], 0.0), "sg")
    op("gp", lambda: nc.gpsimd.memset(am8[:, :, NN:NN2], 0.0), "sg")
    MEMSETS = ("sg", cnt["sg"])
    ESHIFT = op("dve", lambda: nc.vector.memset(eshift, EXP_SHIFT), "sv")

    mark = {}      # generic mark table keyed by (stage, idx)

    # =====================================================================
    # PSUM tenancy: pos 0-15 v-blocks, 16-35 conv chunks, 36-51 attn blocks,
    # 52-55 o chunks.  bank = pos % 8.

    def blk_geom(b):
        br, cb = b // 8, b % 8
        return br, cb, 8 * br, cb * 16, b % 8, b % 8   # ..., slot, bank

    # --- v-blocks: per-block conv into vn16 ---
    VEPI_ENG = ["act", "act", "act", "act", "act", "act", "dve", "dve"]

    def emit_vblk(b):
        br, cb, r0, c0, s, _ = blk_geom(b)
        bank = b % 8
        if b % 4 == 0:
            xdeps("pe", r0, r0 + NR)
        if b == 0:
            wait("pe", SDE)
        if b == 8:
            wait("pe", mark[("vepi", 0)])   # banks 0-3 freed (pairs 0,1)
            wait("pe", mark[("vepi", 1)])
        elif b == 12:
            wait("pe", mark[("vepi", 2)])   # banks 4-7 freed (pairs 2,3)
            wait("pe", mark[("vepi", 3)])
        mark[("vmm", b)] = op(
            "pe",
            lambda o=ps[:, bank, 0:NN], l=w_sb["wvt"],
                   r=xsp[:, r0:r0 + NR, c0:c0 + NC_]:
                nc.tensor.matmul(o, l, r, start=True, stop=True),
            "sp")

    def emit_vepi(pr):   # pair pr covers blocks 2pr, 2pr+1
        b0 = 2 * pr
        bank = b0 % 8
        eng = VEPI_ENG[pr]
        wait(eng, SDE)
        wait(eng, mark[("vmm", b0 + 1)])
        wait(eng, MEMSETS)
        fn = (nc.scalar.activation if eng == "act" else None)
        if eng == "act":
            mark[("vepi", pr)] = op(
                "act",
                lambda o=vn16[:, b0:b0 + 2, 0:NN],
                       i_=ps[:, bank:bank + 2, 0:NN], b_=b_sb["bv"]:
                    nc.scalar.activation(o, i_, RELU, bias=b_),
                "sa")
        else:
            mark[("vepi", pr)] = op(
                "dve",
                lambda o=vn16[:, b0:b0 + 2, 0:NN],
                       i_=ps[:, bank:bank + 2, 0:NN], b_=b_sb["bv"]:
                    nc.vector.tensor_scalar(o, i_, b_, 0.0, ADD, MAXOP),
                "sv")

    def emit_vt_quad(g):   # blocks 4g..4g+3 -> vt16, on sync HWDGE ring
        if g == 0:
            wait("sync", MEMSETS)
        for pr in (2 * g, 2 * g + 1):
            wait("sync", mark[("vepi", pr)])
        mark[("vt", g)] = op(
            "sync",
            lambda o=vt16[:, 4 * g:4 * g + 4], i_=vn16[:, 4 * g:4 * g + 4, :]:
                nc.sync.dma_start(out=o, in_=i_, transpose=True),
            "sdvt", 16)

    for b in range(16):
        emit_vblk(b)
        if b % 2 == 1:
            emit_vepi(b // 2)
        if b % 4 == 3:
            emit_vt_quad(b // 4)

    # --- conv chunks ---
    conv_order = (
        [("k1", j) for j in range(4)] + [("q1", 0), ("q1", 1)]
        + [("k2", j) for j in range(4)] + [("q2", 0), ("q2", 1)]
        + [("k1", 4), ("k1", 5)] + [("k2", 4), ("k2", 5)]
        + [("q1", 2), ("q1", 3)] + [("q2", 2), ("q2", 3)]
    )
    epi_groups = [[0, 1], [2, 3], [4, 5], [6, 7], [8, 9], [10, 11],
                  [12], [13], [14], [15], [16, 17], [18, 19]]
    EPI_ENG = {(0, 1): "act", (2, 3): "dve", (4, 5): "act", (6, 7): "dve",
               (8, 9): "dve", (10, 11): "dve", (12,): "act", (13,): "act",
               (14,): "dve", (15,): "dve", (16, 17): "act", (18, 19): "dve"}
    idx_of = {cj: i for i, cj in enumerate(conv_order)}
    epi_of_idx = {}
    for g in epi_groups:
        for i in g:
            epi_of_idx[i] = tuple(g)

    def chunk_cols(cname, j):
        tot = HALO * W if cname in ("k1", "k2") else NPIX
        return min(CH, tot - j * CH)

    CONV_W = {"k1": "wk1t", "q1": "wq1t", "k2": "wk2t", "q2": "wq2t"}
    CONV_B = {"k1": "bk1", "q1": "bq1", "k2": "bk2", "q2": "bq2"}

    def conv_bank(idx):
        # idx 0-11 rotate banks 0-7; tail idx 12-19 packs banks 4-7 so
        # banks 0-3 free early for attention quad 0
        return idx % 8 if idx < 12 else 4 + idx % 4

    def emit_conv_mm(idx):
        cname, j = conv_order[idx]
        n = chunk_cols(cname, j)
        nrows = n // W
        bank = conv_bank(idx)
        if cname == "k1":
            xdeps("pe", 4 * j, 4 * j + nrows)
            rhs = xsp[:, 4 * j:4 * j + nrows, PAD:PAD + W]
        elif cname == "q1":
            xdeps("pe", PAD + 4 * j, PAD + 4 * j + nrows)
            rhs = xsp[:, PAD + 4 * j:PAD + 4 * j + nrows, PAD:PAD + W]
        elif cname == "k2":
            wait("pe", mark[("cepi", epi_of_idx[idx_of[("k1", j)]])])
            rhs = k1[:, bass.ds(j * CH, n)]
        else:
            wait("pe", mark[("cepi", epi_of_idx[idx_of[("q1", j)]])])
            rhs = q1[:, bass.ds(j * CH, n)]
        # bank free
        if idx < 8:
            wait("pe", mark[("vepi", (8 + idx) // 2)])
        elif idx < 16:
            wait("pe", mark[("cepi", epi_of_idx[idx - 8])])
        else:
            wait("pe", mark[("cepi", epi_of_idx[idx - 4])])
        mark[("cmm", idx)] = op(
            "pe",
            lambda o=ps[:, bank, :n], l=w_sb[CONV_W[cname]], r=rhs:
                nc.tensor.matmul(o, l, r, start=True, stop=True),
            "sp")

    def emit_conv_epi(g):
        idx0 = g[0]
        cname, j0 = conv_order[idx0]
        eng = EPI_ENG[tuple(g)]
        nblk = len(g)
        bank0 = conv_bank(idx0)
        wait(eng, SDE)
        wait(eng, mark[("cmm", g[-1])])
        if eng == "dve":
            wait(eng, MEMSETS)
        b_ap = b_sb[CONV_B[cname]]
        ncols = sum(chunk_cols(cname, conv_order[i][1]) for i in g)
        r0 = 4 * j0
        nrows = ncols // W
        src_ap = (ps[:, bank0:bank0 + 2, :] if nblk == 2
                  else ps[:, bank0, :ncols])
        if nblk == 2:
            src_ap = src_ap.rearrange("p a b -> p (a b)")
        if cname == "k1":
            dst = k1[:, bass.ds(j0 * CH, ncols)]
        elif cname == "q1":
            dst = q1[:, bass.ds(j0 * CH, ncols)]
        elif cname == "q2":
            br_ = j0 // 2
            dst = q[:, br_ * 8:(br_ + 1) * 8, :].rearrange(
                "p b (r w) -> p r b w", w=BC)
            src_ap = src_ap.rearrange("p (r b w) -> p r b w", r=8, w=BC)
        else:   # k2 -> kpad interior
            dst = kpad[:, r0:r0 + nrows, PAD:PAD + W]
            src_ap = src_ap.rearrange("p (r w) -> p r w", w=W)
        if eng == "act":
            mark[("cepi", tuple(g))] = op(
                "act",
                lambda o=dst, i_=src_ap, b_=b_ap:
                    nc.scalar.activation(o, i_, RELU, bias=b_),
                "sa")
        else:
            mark[("cepi", tuple(g))] = op(
                "dve",
                lambda o=dst, i_=src_ap, b_=b_ap:
                    nc.vector.tensor_scalar(o, i_, b_, 0.0, ADD, MAXOP),
                "sv")

    gi = 0
    for idx in range(20):
        emit_conv_mm(idx)
        while gi < len(epi_groups) and epi_groups[gi][-1] <= idx:
            emit_conv_epi(epi_groups[gi])
            gi += 1

    # =====================================================================
    # Attention
    def emit_S(b):
        br, cb, r0, c0, s, bank = blk_geom(b)
        mark[("smm", b)] = op(
            "pe",
            lambda o=ps[:, bank, 0:NN], l=q[:, b, :],
                   r=kpad[:, r0:r0 + NR, c0:c0 + NC_]:
                nc.tensor.matmul(o, l, r, start=True, stop=False),
            "sp")
        mark[("mmm", b)] = op(
            "pe",
            lambda o=ps[:, bank, 0:NN], l=w_sb["id"], r=maskM[:, b, :]:
                nc.tensor.matmul(o, l, r, start=False, stop=True),
            "sp")

    def emit_exp_pair(b0):   # blocks b0, b0+1 in one ACT op (no accum)
        s0 = b0 % 8
        bank0 = b0 % 8
        if b0 == 0:
            wait("act", ESHIFT)
        wait("act", mark[("mmm", b0 + 1)])
        if b0 >= 8:
            wait("act", mark[("norm", b0 - 7)])   # e8 slots free
        mark[("exp", b0)] = op(
            "act",
            lambda o=e8[:, s0:s0 + 2, :], i_=ps[:, bank0:bank0 + 2, 0:NN],
                   sh=eshift:
                nc.scalar.activation(o, i_, EXP, bias=sh),
            "sa")

    def emit_softmax_pair(b0):   # z reduce + oob + recip + norms x2
        s0 = b0 % 8
        g = b0 // 4
        pr = (b0 % 4) // 2
        wait("dve", mark[("exp", b0)])
        if b0 == 0:
            wait("dve", SDW)
        zm = op("dve",
                lambda o=z16[:, b0:b0 + 2], i_=e8[:, s0:s0 + 2, :]:
                    nc.vector.reduce_sum(o, i_, axis=mybir.AxisListType.X),
                "sv")
        # same-engine RAW: consecutive DVE ops pipeline; dependent reads
        # need an explicit sem wait for the writer's completion
        wait("dve", zm)
        zm = op("dve",
                lambda o=z16[:, b0:b0 + 2], i_=z16[:, b0:b0 + 2],
                       i1=oobc16[:, b0:b0 + 2]:
                    nc.vector.tensor_add(o, i_, i1),
                "sv")
        wait("dve", zm)
        rm = op("dve",
                lambda o=rz16[:, b0:b0 + 2], i_=z16[:, b0:b0 + 2]:
                    nc.vector.reciprocal(o, i_),
                "sv")
        wait("dve", rm)
        if g >= 2:
            # am8 slots re-read by the at-half of quad g-2, pair pr
            wait("dve", ("sdat", 16 * (2 * (g - 2) + pr + 1)))
        for b in (b0, b0 + 1):
            s = b % 8
            mark[("norm", b)] = op(
                "dve",
                lambda o=am8[:, s, 0:NN], i_=e8[:, s, :],
                       sc=rz16[:, b:b + 1]:
                    nc.vector.tensor_scalar_mul(o, i_, sc),
                "sv")

    def emit_at_half(b0):   # 2 blocks per transpose, sync ring
        s0 = b0 % 8
        g = b0 // 4
        wait("sync", mark[("norm", b0 + 1)])
        if b0 == 0:
            wait("sync", MEMSETS)
        if g >= 2:
            wait("sync", mark[("av", b0 - 7)])
        mark[("at", b0)] = op(
            "sync",
            lambda o=at8[:, s0:s0 + 2], i_=am8[:, s0:s0 + 2, :]:
                nc.sync.dma_start(out=o, in_=i_, transpose=True),
            "sdat", 16)

    def emit_av(b):
        br, cb, r0, c0, s, bank = blk_geom(b)
        for ch in range(3):
            mark[("av", b)] = op(
                "pe",
                lambda o=ps[:, bank, NN2:CH], l=vt16[:, b, ch, :],
                       r=at8[:, s, ch, :], st=(ch == 0), sp_=(ch == 2):
                    nc.tensor.matmul(o, l, r, start=st, stop=sp_),
                "sp")

    def emit_acopy(b):   # b even, covers b, b+1
        br, cb, r0, c0, s, bank = blk_geom(b)
        wait("act", mark[("av", b + 1)])
        mark[("acopy", b)] = op(
            "act",
            lambda o=attn[:, r0:r0 + BR, c0:c0 + 2 * BC].rearrange(
                       "p r (a w) -> p a r w", w=BC),
                   i_=ps[:, bank:bank + 2, NN2:CH].rearrange(
                       "p a (r w) -> p a r w", w=BC):
                nc.scalar.copy(o, i_),
            "sa")

    # quad-granular PE batches: waits once per quad, then unbroken MM runs
    # so the PE reorder window can prefetch LDWEIGHTS and pipeline drains.
    # Softmax/transpose run per PAIR of blocks to shorten the chain that
    # gates each AV pair.
    def emit_S_quad(g):
        b0 = 4 * g
        br = b0 // 8
        if g == 0:
            wait("pe", SDW)
        wait("pe", mark[("cepi", epi_of_idx[idx_of[("k2", 3 if br == 0 else 5)]
                         ])])
        wait("pe", mark[("cepi", epi_of_idx[idx_of[("q2", 1 if br == 0 else 3)]
                         ])])
        if g == 0:
            for i in (8, 10):      # conv tenants of banks 0-3
                wait("pe", mark[("cepi", epi_of_idx[i])])
        elif g == 1:
            for i in (16, 18):     # conv tail tenants of banks 4-7
                wait("pe", mark[("cepi", epi_of_idx[i])])
        else:
            wait("pe", mark[("acopy", 4 * (g - 2) + 2)])
        for b in range(b0, b0 + 4):
            emit_S(b)
        for p in (0, 2):
            emit_exp_pair(b0 + p)
            emit_softmax_pair(b0 + p)
            emit_at_half(b0 + p)

    def emit_AV_pair(b0):
        g, pr = b0 // 4, (b0 % 4) // 2
        wait("pe", ("sdvt", 16 * (g + 1)))
        wait("pe", ("sdat", 16 * (2 * g + pr + 1)))
        emit_av(b0)
        emit_av(b0 + 1)
        emit_acopy(b0)

    # Output conv: COLUMN chunks (cols 32i..32i+32 over all 16 rows) so each
    # chunk only needs one acopy pair per block-row; host re-interleaves.
    # o_i uses bank i (freed by acopy(8+2*(i and (i-1)))...), interleaved
    # between the last AV pairs.
    def emit_o(i):
        bank = i
        wait("pe", mark[("acopy", 8 + 2 * i)])
        op("pe",
           lambda o=ps[:, bank, :], l=w_sb["wat"],
                  r=attn[:, :, 32 * i:32 * i + 32]:
               nc.tensor.matmul(o, l, r, start=True, stop=False),
           "sp")
        om = op("pe",
                lambda o=ps[:, bank, :], l=w_sb["wxt"],
                       r=xsp[:, PAD:PAD + RPC, PAD + 32 * i:PAD + 32 * i + 32]:
                    nc.tensor.matmul(o, l, r, start=False, stop=True),
                "sp")
        wait("act", om)
        if i >= 2:
            wait("act", mark[("odma", i - 2)])
        mark[("oepi", i)] = op(
            "act",
            lambda o=yt[:, i % 2, :], i_=ps[:, bank, :], b_=b_sb["bo"]:
                nc.scalar.activation(o, i_, IDENT, bias=b_),
            "sa")
        wait("sync", mark[("oepi", i)])
        mark[("odma", i)] = op(
            "sync",
            lambda o=y_d[:, bass.ts(i, CH)], i_=yt[:, i % 2, :]:
                nc.sync.dma_start(out=o, in_=i_),
            "sdout", 16)

    emit_S_quad(0)
    emit_S_quad(1)
    emit_AV_pair(0)
    emit_AV_pair(2)
    emit_S_quad(2)
    emit_AV_pair(4)
    emit_AV_pair(6)
    emit_S_quad(3)
    emit_AV_pair(8)
    emit_o(0)
    emit_AV_pair(10)
    emit_o(1)
    emit_AV_pair(12)
    emit_o(2)
    emit_AV_pair(14)
    emit_o(3)

    if DEBUG_OUTS:
        dbg = {
            "d_q": q.rearrange("p a b -> p (a b)"),
            "d_kpad": kpad.rearrange("p r w -> p (r w)"),
            "d_vn": vn16.rearrange("p a b -> p (a b)"),
            "d_vt": vt16.rearrange("p a b c -> p (a b c)"),
            "d_attn": attn.rearrange("p r w -> p (r w)"),
            "d_z": z16,
            "d_rz": rz16,
            "d_am": am8.rearrange("p a b -> p (a b)"),
            "d_at": at8.rearrange("p a b c -> p (a b c)"),
        }
        for nm, src in dbg.items():
            dd = nc.dram_tensor(nm, list(src.shape),
                                src.dtype, kind="ExternalOutput").ap()
            for s_ in ("sp", "sa", "sv"):
                wait("sync", (s_, cnt[s_]))
            op("sync", lambda o=dd, i_=src: nc.sync.dma_start(out=o, in_=i_),
               "sdout", 16)

    # ---- tail barrier ----
    for s_ in ("sp", "sa", "sv", "sg", "sdvt", "sdat", "sdath", "sdout",
               "sde", "sdw"):
        wait("sync", (s_, cnt[s_]))
    for j in range(6):
        wait("sync", (f"sdx{j}", cnt[f"sdx{j}"]))

    # ---- emit ----
    def run(eng_name, eng_obj):
        hwm = {}
        for item in plan[eng_name]:
            if item[0] == "w":
                _, s_, v = item
                if hwm.get(s_, 0) >= v:
                    continue
                hwm[s_] = v
                eng_obj.wait_ge(sems[s_], v)
            else:
                _, fn, s_, inc = item
                inst = fn()
                if s_:
                    inst.then_inc(sems[s_], inc)

    with nc.Block() as block:
        @block.sync
        def _(e):
            run("sync", e)

        @block.tensor
        def _(e):
            run("pe", e)

        @block.scalar
        def _(e):
            run("act", e)

        @block.vector
        def _(e):
            run("dve", e)

        @block.gpsimd
        def _(e):
            run("gp", e)

    # no explicit sem-clear block: walrus's block-boundary machinery
    # already resets all semaphores (a second Block costs ~7us of
    # per-semaphore clears on every engine).
    return nc


_PROGRAM = None


def _host_inputs(x, w_q1, s_q1, b_q1, w_q2, s_q2, b_q2,
                 w_k1, s_k1, b_k1, w_k2, s_k2, b_k2,
                 w_v, s_v, b_v, w_o, s_o, b_o):
    def foldT(w, s):
        return np.ascontiguousarray((s[:, None] * w).T.astype(ml_dtypes.bfloat16))

    wq1t, wq2t = foldT(w_q1, s_q1), foldT(w_q2, s_q2)
    wk1t, wk2t = foldT(w_k1, s_k1), foldT(w_k2, s_k2)
    wvt = foldT(w_v, s_v)
    wo = s_o[:, None] * w_o
    wat = np.ascontiguousarray(wo[:, :C].T.astype(ml_dtypes.bfloat16))
    wxt = np.ascontiguousarray(wo[:, C:].T.astype(ml_dtypes.bfloat16))

    col = lambda b: np.ascontiguousarray(b.astype(np.float32)[:, None])

    # window-validity over the 14x22 neighborhood, per block pixel
    valid = np.zeros((BR * BC, NR, NC_), bool)
    for r in range(BR):
        for c in range(BC):
            p = r * BC + c
            valid[p, r:r + 7, c:c + 7] = True

    X = np.asarray(x, np.float32).reshape(C, H, W)
    wall = np.concatenate(
        [wq1t, wq2t, wk1t, wk2t, wvt, wat, wxt,
         np.eye(C, dtype=ml_dtypes.bfloat16)], axis=1)
    shared = dict(wall=np.ascontiguousarray(wall))

    e16v = np.float32(np.exp(EXP_SHIFT))
    in_maps = []
    for core in range(NCORES):
        h0 = core * RPC
        xsb = np.zeros((C, HALO, WP), np.float32)
        lo, hi = h0 - PAD, h0 + RPC + PAD
        slo, shi = max(lo, 0), min(hi, H)
        xsb[:, slo - lo:shi - lo, PAD:PAD + W] = X[:, slo:shi]

        # per-block additive mask (0 = in-window & in-image; MASKV else)
        # and oob compensation = (# window positions outside the image)*e^-16.
        # neighborhood row index ri -> image row h0 + br*8 + ri - 3
        # neighborhood col index ci -> image col cb*16 + ci - 3
        maskm = np.empty((16, BR * BC, NN), np.float32)
        oobc = np.empty((16, BR * BC), np.float32)
        for b in range(16):
            brr, cb = b // 8, b % 8
            rowok = np.array([0 <= h0 + brr * BR + ri - PAD < H
                              for ri in range(NR)])
            colok = np.array([0 <= cb * BC + ci - PAD < W
                              for ci in range(NC_)])
            inimg = rowok[:, None] & colok[None, :]
            mb = np.where(valid & inimg[None, :, :], 0.0, MASKV)
            maskm[b] = mb.reshape(BR * BC, NN)
            # per pixel: count of its 49 window positions that are OOB
            n_oob = (valid & ~inimg[None, :, :]).sum(axis=(1, 2))
            oobc[b] = n_oob * e16v
        m = dict(shared)
        m["xs"] = np.ascontiguousarray(
            xsb.reshape(C, NXP).astype(ml_dtypes.bfloat16))
        m["smask"] = np.ascontiguousarray(
            maskm.transpose(1, 0, 2).astype(ml_dtypes.bfloat16))
        m["oobc"] = np.ascontiguousarray(oobc.T.astype(np.float32))
        m["ball"] = np.ascontiguousarray(np.concatenate(
            [col(b_q1), col(b_q2), col(b_k1), col(b_k2), col(b_v),
             col(b_o), np.zeros((C, 2), np.float32)], axis=1))
        in_maps.append(m)
    return in_maps


def kernel(**inputs):
    global _PROGRAM
    if _PROGRAM is None:
        _PROGRAM = _build_program()
    in_maps = _host_inputs(**{k: np.asarray(v) for k, v in inputs.items()})
    res = run_bass_kernel_spmd(_PROGRAM, in_maps, core_ids=list(range(NCORES)))
    # y chunks are column-major: [C, 4 colchunks, 16 rows, 32 cols]
    stripes = [np.asarray(r["y"]).astype(np.float32)
               .reshape(C, 4, RPC, 32).transpose(0, 2, 1, 3).reshape(C, RPC, W)
               for r in res.results]
    return np.concatenate(stripes, axis=1).reshape(1, C, H, W)


if __name__ == "__main__":
    rng = np.random.default_rng(0)
    fake = {"x": rng.standard_normal((1, C, H, W)).astype(np.float32)}
    for n in ("q1", "q2", "k1", "k2", "v", "o"):
        cin = 2 * C if n == "o" else C
        fake["w_" + n] = (rng.standard_normal((C, cin)) / np.sqrt(cin)).astype(np.float32)
        fake["s_" + n] = rng.uniform(0.5, 1.5, C).astype(np.float32)
        fake["b_" + n] = (rng.standard_normal(C) * 0.1).astype(np.float32)
    out = kernel(**fake)
    print("kernel output", out.shape, out.dtype)
